# revision 8
# baseline (speedup 1.0000x reference)
"""Multi-head attention forward (B=4, T=2048, D=1024, H=16), sharded over
8 Trainium2 NeuronCores.

Sharding: data-parallel over batch (4) x query-halves (2). Core c handles
batch b=c//2 and query rows [hf*TQ, (hf+1)*TQ) with hf=c%2, TQ=T//2. Each
core computes K/V over the full sequence for its batch element (duplicated
across the 2 cores of a batch -- cheaper than a cross-core reduce), so the
host-side gather is a pure concatenation.

All on-device layouts are chosen so no transposes are ever needed:
  x^T (pre-transposed on host as part of the sharding layout)
    Q^T[dq,t] = W_q[din,dq].T @ x^T[din,t]        (lhsT = W_q as stored)
    K^T[dk,t] = W_k[din,dk].T @ x^T[din,t]
    V[t,dv]   = x^T[din,t].T @ W_v[din,dv]        (natural layout)
  S^T[k,q] = K^T[dh,k].T @ Q^T[dh,q]              (keys on partitions)
  P^T = Exp(0.125*S^T + maskbias)  -- one fused ACT op; maskbias is the
        additive -1000 mask which varies along k = the partition dim, so it
        rides the per-partition bias operand. No max-subtraction: scores are
        N(0,1)-scaled so exp never overflows fp32, and masked lanes hit
        exp(~-1000) = 0 exactly, matching the reference softmax.
  Z via a ones-column appended to V: out'^T[0:64] = V^T P^T, out'^T[64] = Z
  att^T = out'^T[0:64] * (1/Z) (partition-broadcast via stride-0 DMA)
  y[t,dc] = att^T[din,t].T @ W_out[din,dc] + b_out (natural layout -> DMA out)
"""

import os
import sys
import types
from contextlib import ExitStack

import numpy as np
import ml_dtypes

import concourse.bass as bass
import concourse.tile as tile
import concourse.mybir as mybir
from concourse import bacc, bass_utils

P = 128
DH = 64

F32 = mybir.dt.float32
BF16 = mybir.dt.bfloat16
F32R = mybir.dt.float32r

# Full-problem dims (hardcoded per contract).
FULL_DIMS = dict(B=4, T=2048, D=1024, H=16)

DEFAULT_CFG = dict(
    dt_x=BF16,      # xT / xqT storage (dram + sbuf)
    dt_w=BF16,      # W_qkv streaming blocks
    dt_kv=BF16,     # K^T and V(aug) sbuf storage; must equal dt_p
    dt_q=BF16,      # Q^T sbuf storage
    dt_p=BF16,      # P^T (softmax numerator) sbuf storage
    dt_att=BF16,    # att^T and W_out storage
    use_f32r=True,  # bitcast fp32 matmul operands to float32r (4x faster)
    st_bufs=3,
    pt_bufs=4,
    ot_bufs=2,
)


def _np_dt(dt):
    return {F32: np.float32, BF16: ml_dtypes.bfloat16}[dt]


def _install_ntff_shim():
    """The agent image's antenv lacks axon_hooks; bass_utils needs it for
    trace=True under axon. Provide it from the boot module."""
    if "antenv.axon_hooks" in sys.modules:
        return
    try:
        from trn_agent_boot.trn_boot import _ntff_profile_via_ctypes
        hook = _ntff_profile_via_ctypes("/opt/axon/libaxon_pjrt.so")
    except Exception:
        hook = None
    mod = types.ModuleType("antenv.axon_hooks")
    mod.get_axon_ntff_profile_hook = lambda: hook
    mod.set_axon_ntff_profile_hook = lambda h: None
    sys.modules["antenv.axon_hooks"] = mod


def _bcast_ap(ap, n):
    """[1, N] sbuf AP -> [n, N] partition-broadcast read AP (step 0)."""
    return bass.AP(
        tensor=ap.tensor,
        offset=ap.offset,
        ap=[[0, n]] + [list(p) for p in ap.ap[1:]],
    )


def build_nc(dims, cfg):
    """Build the per-core SPMD program. Returns (nc, input_names)."""
    T, D, H = dims["T"], dims["D"], dims["H"]
    assert H * DH == D
    TQ = T // 2           # queries per core
    NDIN = D // P         # contraction tiles for the projections
    NK = T // P           # key tiles
    QB = min(512, TQ)     # q-block (psum free width)
    NQB = TQ // QB
    NHT = H // 2          # head-pair tiles (2 heads of 64 per 128 partitions)
    TH = min(1024, T)     # t-chunk for phase-1 K/V streaming
    NTH = T // TH
    FB = min(512, TH)     # free-block for K^T compute
    FBQ = min(512, TQ)    # free-block for Q^T compute
    FBV = min(512, D)     # dv-block for V compute
    FBO = min(512, D)     # dc-block for out projection
    D3 = 3 * D

    dt_x, dt_w = cfg["dt_x"], cfg["dt_w"]
    dt_kv, dt_q, dt_p, dt_att = cfg["dt_kv"], cfg["dt_q"], cfg["dt_p"], cfg["dt_att"]
    assert dt_p == dt_kv, "PV matmul needs matching operand dtypes"

    def mm(ap):
        if cfg["use_f32r"] and ap.dtype == F32:
            return ap.bitcast(F32R)
        return ap

    nc = bacc.Bacc("TRN2", target_bir_lowering=False, debug=False)

    xT_d = nc.dram_tensor("xT", [D, T], dt_x, kind="ExternalInput")
    xqT_d = nc.dram_tensor("xqT", [D, TQ], dt_x, kind="ExternalInput")
    wqkv_d = nc.dram_tensor("wqkv", [D, D3], dt_w, kind="ExternalInput")
    wout_d = nc.dram_tensor("wout", [D, D], dt_att, kind="ExternalInput")
    bq_d = nc.dram_tensor("bq", [P, NDIN], F32, kind="ExternalInput")
    bk_d = nc.dram_tensor("bk", [P, NDIN], F32, kind="ExternalInput")
    bv_d = nc.dram_tensor("bv", [P, D], F32, kind="ExternalInput")
    bo_d = nc.dram_tensor("bo", [P, D], F32, kind="ExternalInput")
    maskm_d = nc.dram_tensor("maskm", [P, NK], F32, kind="ExternalInput")
    y_d = nc.dram_tensor("y", [TQ, D], F32, kind="ExternalOutput")

    in_names = ["xT", "xqT", "wqkv", "wout", "bq", "bk", "bv", "bo", "maskm"]

    # wqkv viewed as [p, din_tile, col] so one DMA grabs a column block
    # across all NDIN din tiles.
    wqkv_v = wqkv_d.ap().rearrange("(j p) n -> p j n", p=P)
    wout_v = wout_d.ap().rearrange("(j p) n -> p j n", p=P)

    IDENT = mybir.ActivationFunctionType.Identity
    EXP = mybir.ActivationFunctionType.Exp

    with tile.TileContext(nc) as tc, ExitStack() as stk:
        misc = stk.enter_context(tc.tile_pool(name="misc", bufs=1))
        pers = stk.enter_context(tc.tile_pool(name="pers", bufs=1))

        # --- small persistent tiles ----------------------------------------
        bq_sb = misc.tile([P, NDIN], F32, tag="bq", name="bq_sb")
        nc.sync.dma_start(out=bq_sb, in_=bq_d.ap())
        bk_sb = misc.tile([P, NDIN], F32, tag="bk", name="bk_sb")
        nc.sync.dma_start(out=bk_sb, in_=bk_d.ap())
        bv_sb = misc.tile([P, D], F32, tag="bv", name="bv_sb")
        nc.sync.dma_start(out=bv_sb, in_=bv_d.ap())
        bo_sb = misc.tile([P, D], F32, tag="bo", name="bo_sb")
        nc.sync.dma_start(out=bo_sb, in_=bo_d.ap())

        mf_sb = misc.tile([P, NK], F32, tag="mf", name="mf_sb")
        nc.sync.dma_start(out=mf_sb, in_=maskm_d.ap())
        m1_sb = misc.tile([P, NK], F32, tag="m1", name="m1_sb")
        nc.vector.tensor_scalar_add(m1_sb, mf_sb, -1.0)
        maskadd = misc.tile([P, NK], F32, tag="maskadd", name="maskadd")
        nc.vector.tensor_scalar_mul(maskadd, m1_sb, 1000.0)

        # --- persistent big tensors ----------------------------------------
        KT = [pers.tile([P, T], dt_kv, tag=f"KT{i}", name=f"KT{i}")
              for i in range(NDIN)]
        QT = [pers.tile([P, TQ], dt_q, tag=f"QT{i}", name=f"QT{i}")
              for i in range(NDIN)]
        VA = [pers.tile([P, H * (DH + 1)], dt_kv, tag=f"VA{i}", name=f"VA{i}")
              for i in range(NK)]
        ATT = [pers.tile([P, TQ], dt_att, tag=f"ATT{i}", name=f"ATT{i}")
               for i in range(NDIN)]

        # ones columns of the augmented V
        for kt in range(NK):
            va_v = VA[kt].rearrange("p (h c) -> p h c", c=DH + 1)
            nc.vector.memset(va_v[:, :, DH:DH + 1], 1.0)

        # ================= Phase 1: QKV projections ========================
        with tc.tile_pool(name="ph1", bufs=1) as ph1, \
             tc.tile_pool(name="wstr", bufs=1) as wstr, \
             tc.tile_pool(name="p1ps", bufs=1, space="PSUM") as p1ps:

            for th in range(NTH):
                xth = []
                for j in range(NDIN):
                    xt = ph1.tile([P, TH], dt_x, tag=f"xth{j}",
                                  name=f"xth{j}_{th}")
                    nc.sync.dma_start(
                        out=xt,
                        in_=xT_d.ap()[j * P:(j + 1) * P,
                                      th * TH:(th + 1) * TH])
                    xth.append(xt)

                # K^T for this chunk
                for dk in range(NDIN):
                    wb = wstr.tile([P, NDIN, P], dt_w, tag="wblk", bufs=3,
                                   name=f"wbk{th}_{dk}")
                    nc.sync.dma_start(
                        out=wb, in_=wqkv_v[:, :, D + dk * P:D + (dk + 1) * P])
                    for fb in range(TH // FB):
                        ps = p1ps.tile([P, FB], F32, tag="kps", bufs=2,
                                       name=f"kps{th}_{dk}_{fb}")
                        for j in range(NDIN):
                            nc.tensor.matmul(
                                ps, mm(wb[:, j, :]),
                                mm(xth[j][:, fb * FB:(fb + 1) * FB]),
                                start=(j == 0), stop=(j == NDIN - 1))
                        nc.scalar.activation(
                            out=KT[dk][:, th * TH + fb * FB:
                                       th * TH + (fb + 1) * FB],
                            in_=ps, func=IDENT,
                            bias=bk_sb[:, dk:dk + 1], scale=1.0)

                # V for this chunk (natural layout, scattered into VA)
                hpb = FBV // DH  # heads per dv block
                for dv2 in range(D // FBV):
                    wv = wstr.tile([P, NDIN, FBV], dt_w, tag="wv", bufs=2,
                                   name=f"wv{th}_{dv2}")
                    nc.sync.dma_start(
                        out=wv,
                        in_=wqkv_v[:, :, 2 * D + dv2 * FBV:
                                   2 * D + (dv2 + 1) * FBV])
                    for tt in range(TH // P):
                        kt = (th * TH) // P + tt
                        ps = p1ps.tile([P, FBV], F32, tag="vps", bufs=2,
                                       name=f"vps{th}_{dv2}_{tt}")
                        for j in range(NDIN):
                            nc.tensor.matmul(
                                ps, mm(xth[j][:, tt * P:(tt + 1) * P]),
                                mm(wv[:, j, :]),
                                start=(j == 0), stop=(j == NDIN - 1))
                        va_v = VA[kt].rearrange("p (h c) -> p h c", c=DH + 1)
                        nc.vector.tensor_add(
                            va_v[:, dv2 * hpb:(dv2 + 1) * hpb, 0:DH],
                            ps.rearrange("p (h c) -> p h c", c=DH),
                            bv_sb[:, dv2 * FBV:(dv2 + 1) * FBV]
                            .rearrange("p (h c) -> p h c", c=DH))

            # Q^T
            xqs = []
            for j in range(NDIN):
                xq = ph1.tile([P, TQ], dt_x, tag=f"xq{j}", name=f"xq{j}")
                nc.sync.dma_start(out=xq, in_=xqT_d.ap()[j * P:(j + 1) * P, :])
                xqs.append(xq)
            for dq in range(NDIN):
                wb = wstr.tile([P, NDIN, P], dt_w, tag="wblk", bufs=3,
                               name=f"wbq{dq}")
                nc.sync.dma_start(
                    out=wb, in_=wqkv_v[:, :, dq * P:(dq + 1) * P])
                for fb in range(TQ // FBQ):
                    ps = p1ps.tile([P, FBQ], F32, tag="kps", bufs=2,
                                   name=f"qps{dq}_{fb}")
                    for j in range(NDIN):
                        nc.tensor.matmul(
                            ps, mm(wb[:, j, :]),
                            mm(xqs[j][:, fb * FBQ:(fb + 1) * FBQ]),
                            start=(j == 0), stop=(j == NDIN - 1))
                    nc.scalar.activation(
                        out=QT[dq][:, fb * FBQ:(fb + 1) * FBQ],
                        in_=ps, func=IDENT,
                        bias=bq_sb[:, dq:dq + 1], scale=1.0)

        # ================= Phase 2: attention ==============================
        with tc.tile_pool(name="ph2", bufs=1) as ph2:
            wout_sb = []
            for j in range(NDIN):
                wo = ph2.tile([P, D], dt_att, tag=f"wo{j}", name=f"wo{j}")
                nc.sync.dma_start(out=wo, in_=wout_v[:, j, :])
                wout_sb.append(wo)

            zdram = stk.enter_context(
                tc.tile_pool(name="zdram", bufs=1, space="DRAM"))

            # Unnormalized att^T and the Z rows, collected over the whole
            # attention sweep; Z reciprocals are batched into one wide DVE
            # op afterwards ([1,512] reciprocals cost 3.3us each -- a
            # single-partition DVE op -- so per-(head,qb) recips serialize).
            OTU = [ph2.tile([P, TQ], F32, tag=f"OTU{i}", name=f"OTU{i}")
                   for i in range(NDIN)]
            zall = ph2.tile([2 * NHT, TQ], F32, tag="zall", name="zall")
            zinv = ph2.tile([2 * NHT, TQ], F32, tag="zinv", name="zinv")

            with tc.tile_pool(name="p2ps", bufs=1, space="PSUM") as p2ps:
                for hp in range(NHT):
                    for qb in range(NQB):
                        ot = [p2ps.tile([DH + 1, QB], F32, tag=f"ot{s}",
                                        bufs=cfg["ot_bufs"],
                                        name=f"ot{s}_{hp}_{qb}")
                              for s in range(2)]
                        for kt in range(NK):
                            st = []
                            # the two heads' S^T matmuls use disjoint 64-row
                            # groups of the PE array; emitted back-to-back
                            # they overlap almost fully.
                            for s in range(2):
                                base = s * DH
                                stt = p2ps.tile([P, QB], F32, tag="st",
                                                bufs=cfg["st_bufs"],
                                                name=f"st{hp}_{qb}_{kt}_{s}")
                                nc.tensor.matmul(
                                    stt,
                                    mm(KT[hp][base:base + DH,
                                              kt * P:(kt + 1) * P]),
                                    mm(QT[hp][base:base + DH,
                                              qb * QB:(qb + 1) * QB]),
                                    start=True, stop=True)
                                st.append(stt)
                            pt = []
                            for s in range(2):
                                ptt = ph2.tile([P, QB], dt_p, tag="pt",
                                               bufs=cfg["pt_bufs"],
                                               name=f"pt{hp}_{qb}_{kt}_{s}")
                                nc.scalar.activation(
                                    out=ptt, in_=st[s], func=EXP,
                                    bias=maskadd[:, kt:kt + 1], scale=0.125)
                                pt.append(ptt)
                            for s in range(2):
                                h = 2 * hp + s
                                nc.tensor.matmul(
                                    ot[s],
                                    mm(VA[kt][:, h * (DH + 1):
                                              (h + 1) * (DH + 1)]),
                                    mm(pt[s]),
                                    start=(kt == 0), stop=(kt == NK - 1))
                        for s in range(2):
                            base = s * DH
                            qsl = slice(qb * QB, (qb + 1) * QB)
                            nc.vector.tensor_copy(
                                OTU[hp][base:base + DH, qsl], ot[s][0:DH, :])
                            # engine writes must start at a 32-aligned
                            # partition; stage Z at partition 0, then DMA
                            # (no alignment constraint) into the packed row.
                            zt = ph2.tile([1, QB], F32, tag="zt", bufs=4,
                                          name=f"zt{hp}_{qb}_{s}")
                            nc.scalar.copy(zt, ot[s][DH:DH + 1, :])
                            nc.sync.dma_start(
                                out=zall[2 * hp + s:2 * hp + s + 1, qsl],
                                in_=zt)

                # batched 1/Z + partition-broadcast (SBUF src DMAs forbid
                # step-0 partition reads, DRAM srcs allow them -- bounce
                # through a DRAM scratch) + normalize into ATT.
                nc.vector.reciprocal(zinv, zall)
                zd = zdram.tile([2 * NHT, TQ], F32, tag="zd", name="zd")
                nc.sync.dma_start(out=zd, in_=zinv)
                for hp in range(NHT):
                    for qb in range(NQB):
                        qsl = slice(qb * QB, (qb + 1) * QB)
                        # both heads' 1/Z rows -> [128, QB] (64x repeat each)
                        # so the normalize mul has equal SBUF base partitions.
                        zsl = zd[2 * hp:2 * hp + 2, qsl]
                        zsrc = bass.AP(
                            tensor=zsl.tensor, offset=zsl.offset,
                            ap=[list(zsl.ap[0]), [0, DH], list(zsl.ap[1])])
                        zr = ph2.tile([P, QB], F32, tag="zr", bufs=4,
                                      name=f"zr{hp}_{qb}")
                        nc.sync.dma_start(out=zr, in_=zsrc)
                        nc.vector.tensor_mul(
                            ATT[hp][:, qsl], OTU[hp][:, qsl], zr)

            # ============= Phase 3: output projection ======================
            with tc.tile_pool(name="ph3", bufs=1) as ph3, \
                 tc.tile_pool(name="p3ps", bufs=1, space="PSUM") as p3ps:
                for tb in range(TQ // P):
                    for dc in range(D // FBO):
                        ps = p3ps.tile([P, FBO], F32, tag="ops", bufs=2,
                                       name=f"ops{tb}_{dc}")
                        for j in range(NDIN):
                            nc.tensor.matmul(
                                ps,
                                mm(ATT[j][:, tb * P:(tb + 1) * P]),
                                mm(wout_sb[j][:, dc * FBO:(dc + 1) * FBO]),
                                start=(j == 0), stop=(j == NDIN - 1))
                        ob = ph3.tile([P, FBO], F32, tag="ob", bufs=3,
                                      name=f"ob{tb}_{dc}")
                        nc.vector.tensor_add(
                            ob, ps, bo_sb[:, dc * FBO:(dc + 1) * FBO])
                        nc.sync.dma_start(
                            out=y_d.ap()[tb * P:(tb + 1) * P,
                                         dc * FBO:(dc + 1) * FBO],
                            in_=ob)

    nc.compile()
    return nc, in_names


def shard_inputs(dims, cfg, x, mask, W_qkv, b_qkv, W_out, b_out):
    """Host-side sharding: slices, layout transposes, bias tiling."""
    B, T, D = dims["B"], dims["T"], dims["D"]
    TQ = T // 2
    NDIN = D // P
    NK = T // P
    npx = _np_dt(cfg["dt_x"])
    npw = _np_dt(cfg["dt_w"])
    npa = _np_dt(cfg["dt_att"])

    x = np.asarray(x)
    mask = np.asarray(mask)
    W_qkv = np.asarray(W_qkv)
    b_qkv = np.asarray(b_qkv)
    W_out = np.asarray(W_out)
    b_out = np.asarray(b_out)

    wqkv_c = np.ascontiguousarray(W_qkv.astype(npw))
    wout_c = np.ascontiguousarray(W_out.astype(npa))
    bq = np.ascontiguousarray(b_qkv[:D].reshape(NDIN, P).T.astype(np.float32))
    bk = np.ascontiguousarray(
        b_qkv[D:2 * D].reshape(NDIN, P).T.astype(np.float32))
    bv = np.ascontiguousarray(
        np.broadcast_to(b_qkv[2 * D:], (P, D)).astype(np.float32))
    bo = np.ascontiguousarray(
        np.broadcast_to(b_out, (P, D)).astype(np.float32))

    in_maps = []
    xT_cache = {}
    for c in range(2 * B):
        b, hf = c // 2, c % 2
        if b not in xT_cache:
            xT_cache[b] = np.ascontiguousarray(x[b].T.astype(npx))
        xqT = np.ascontiguousarray(
            x[b, hf * TQ:(hf + 1) * TQ, :].T.astype(npx))
        maskm = np.ascontiguousarray(
            mask[b, 0, 0].reshape(NK, P).T.astype(np.float32))
        in_maps.append(dict(
            xT=xT_cache[b], xqT=xqT, wqkv=wqkv_c, wout=wout_c,
            bq=bq, bk=bk, bv=bv, bo=bo, maskm=maskm))
    return in_maps


_CACHE = {}
LAST_EXEC_NS = None


def kernel(x, mask, W_qkv, b_qkv, W_out, b_out):
    global LAST_EXEC_NS
    dims = FULL_DIMS
    cfg = DEFAULT_CFG
    _install_ntff_shim()

    key = "full"
    if key not in _CACHE:
        _CACHE[key] = build_nc(dims, cfg)
    nc, _ = _CACHE[key]

    in_maps = shard_inputs(dims, cfg, x, mask, W_qkv, b_qkv, W_out, b_out)
    trace = bool(os.environ.get("KERNEL_TRACE"))
    res = bass_utils.run_bass_kernel_spmd(
        nc, in_maps, core_ids=list(range(8)), trace=trace,
        tmpdir=os.environ.get("KERNEL_TRACE_DIR") or None)
    LAST_EXEC_NS = res.exec_time_ns

    B, T, D = dims["B"], dims["T"], dims["D"]
    TQ = T // 2
    out = np.empty((B, T, D), dtype=np.float32)
    for c in range(2 * B):
        b, hf = c // 2, c % 2
        out[b, hf * TQ:(hf + 1) * TQ, :] = res.results[c]["y"]
    return out


# revision 9
# speedup vs baseline: 1.9980x; 1.9980x over previous
"""Multi-head attention forward (B=4, T=2048, D=1024, H=16), sharded over
8 Trainium2 NeuronCores.

Sharding: data-parallel over batch (4) x query-halves (2). Core c handles
batch b=c//2 and query rows [hf*TQ, (hf+1)*TQ) with hf=c%2, TQ=T//2. Each
core computes K/V over the full (compacted) sequence for its batch element
(duplicated across the 2 cores of a batch -- cheaper than a cross-core
reduce), so the host-side gather is a pure concatenation.

Key compaction: attention is permutation-invariant over key positions, so
the host picks a key ORDER (a layout permutation of x's rows / the mask)
that puts unmasked keys first, and the kernel only touches the first
NKC = ceil(max_unmasked/128) key tiles. Masked/padding keys still flow
through the same on-device mask bias (exp(-1000+s) == 0 in fp32, exactly
like the reference softmax); dropped tiles are all-masked keys whose
softmax weight is exactly 0. The program is compiled per NKC (cached);
the fixed Bernoulli(0.5) mask gives NKC=9 vs 16 full tiles.

All on-device layouts are chosen so no transposes are ever needed:
  x^T (pre-transposed on host as part of the sharding layout)
    Q^T[dq,t] = W_q[din,dq].T @ x^T[din,t]        (lhsT = W_q as stored)
    K^T[dk,t] = W_k[din,dk].T @ x^T[din,t]
    V[t,dv]   = x^T[din,t].T @ W_v[din,dv]        (natural layout)
  S^T[k,q] = K^T[dh,k].T @ Q^T[dh,q]              (keys on partitions)
  P^T = Exp(0.125*S^T + maskbias)  -- one fused ACT op per (head, ktile);
        maskbias varies along k = the partition dim, so it rides the
        per-partition bias operand. No max-subtraction: scores are
        N(0,1)-scaled so exp never overflows fp32.
  Z via a ones-column appended to V: out'^T[0:64] = V^T P^T, out'^T[64] = Z
  att^T = out'^T[0:64] * (1/Z)  (Z reciprocals batched into one wide DVE op;
        partition-broadcast via a stride-0 DRAM read)
  y[t,dc] = att^T[din,t].T @ W_out[din,dc] + b_out (natural layout -> DMA)

Phase 2 is ACT(exp)-throughput-bound, so the emission order software-
pipelines the PE: exp(kt) on ACT runs while PE does S^T(kt+1), then PV(kt).
"""

import os
import sys
import types
from contextlib import ExitStack

import numpy as np
import ml_dtypes

import concourse.bass as bass
import concourse.tile as tile
import concourse.mybir as mybir
from concourse import bacc, bass_utils

P = 128
DH = 64

F32 = mybir.dt.float32
BF16 = mybir.dt.bfloat16
F32R = mybir.dt.float32r

# Full-problem dims (hardcoded per contract).
FULL_DIMS = dict(B=4, T=2048, D=1024, H=16)

DEFAULT_CFG = dict(
    dt_x=BF16,      # xT / xqT storage (dram + sbuf)
    dt_w=BF16,      # W_qkv streaming blocks
    dt_kv=BF16,     # K^T and V(aug) sbuf storage; must equal dt_p
    dt_q=BF16,      # Q^T sbuf storage
    dt_p=BF16,      # P^T (softmax numerator) sbuf storage
    dt_att=BF16,    # att^T and W_out storage
    use_f32r=True,  # bitcast fp32 matmul operands to float32r (4x faster)
    st_bufs=3,
    pt_bufs=4,
)


def _np_dt(dt):
    return {F32: np.float32, BF16: ml_dtypes.bfloat16}[dt]


def _install_ntff_shim():
    """The agent image's antenv lacks axon_hooks; bass_utils needs it for
    trace=True under axon. Provide it from the boot module."""
    if "antenv.axon_hooks" in sys.modules:
        return
    try:
        from trn_agent_boot.trn_boot import _ntff_profile_via_ctypes
        hook = _ntff_profile_via_ctypes("/opt/axon/libaxon_pjrt.so")
    except Exception:
        hook = None
    mod = types.ModuleType("antenv.axon_hooks")
    mod.get_axon_ntff_profile_hook = lambda: hook
    mod.set_axon_ntff_profile_hook = lambda h: None
    sys.modules["antenv.axon_hooks"] = mod


def _chunks(total, sz):
    out, off = [], 0
    while off < total:
        c = min(sz, total - off)
        out.append((off, c))
        off += c
    return out


def build_nc(dims, cfg, NKC):
    """Build the per-core SPMD program for NKC compacted key tiles."""
    T, D, H = dims["T"], dims["D"], dims["H"]
    assert H * DH == D
    TQ = T // 2           # queries per core
    NDIN = D // P         # contraction tiles for the projections
    TKC = NKC * P         # compacted key positions
    FBV = min(512, D)     # dv-block for V compute
    FBO = min(512, D)     # dc-block for out projection
    D3 = 3 * D

    dt_x, dt_w = cfg["dt_x"], cfg["dt_w"]
    dt_kv, dt_q, dt_p, dt_att = cfg["dt_kv"], cfg["dt_q"], cfg["dt_p"], cfg["dt_att"]
    assert dt_p == dt_kv, "PV matmul needs matching operand dtypes"

    def mm(ap):
        if cfg["use_f32r"] and ap.dtype == F32:
            return ap.bitcast(F32R)
        return ap

    nc = bacc.Bacc("TRN2", target_bir_lowering=False, debug=False)

    xkT_d = nc.dram_tensor("xkT", [D, TKC], dt_x, kind="ExternalInput")
    xqT_d = nc.dram_tensor("xqT", [D, TQ], dt_x, kind="ExternalInput")
    wqkv_d = nc.dram_tensor("wqkv", [D, D3], dt_w, kind="ExternalInput")
    wout_d = nc.dram_tensor("wout", [D, D], dt_att, kind="ExternalInput")
    bq_d = nc.dram_tensor("bq", [P, NDIN], F32, kind="ExternalInput")
    bk_d = nc.dram_tensor("bk", [P, NDIN], F32, kind="ExternalInput")
    bv_d = nc.dram_tensor("bv", [P, D], F32, kind="ExternalInput")
    bo_d = nc.dram_tensor("bo", [P, D], F32, kind="ExternalInput")
    maskm_d = nc.dram_tensor("maskm", [P, NKC], F32, kind="ExternalInput")
    y_d = nc.dram_tensor("y", [TQ, D], F32, kind="ExternalOutput")

    in_names = ["xkT", "xqT", "wqkv", "wout", "bq", "bk", "bv", "bo", "maskm"]

    # wqkv viewed as [p, din_tile, col] so one DMA grabs a column block
    # across all NDIN din tiles.
    wqkv_v = wqkv_d.ap().rearrange("(j p) n -> p j n", p=P)
    wout_v = wout_d.ap().rearrange("(j p) n -> p j n", p=P)

    EXP = mybir.ActivationFunctionType.Exp

    with tile.TileContext(nc) as tc, ExitStack() as stk:
        misc = stk.enter_context(tc.tile_pool(name="misc", bufs=1))
        pers = stk.enter_context(tc.tile_pool(name="pers", bufs=1))
        zdram = stk.enter_context(
            tc.tile_pool(name="zdram", bufs=1, space="DRAM"))

        # --- small persistent tiles ----------------------------------------
        bq_sb = misc.tile([P, NDIN], F32, tag="bq", name="bq_sb")
        nc.sync.dma_start(out=bq_sb, in_=bq_d.ap())
        bk_sb = misc.tile([P, NDIN], F32, tag="bk", name="bk_sb")
        nc.sync.dma_start(out=bk_sb, in_=bk_d.ap())
        bv_sb = misc.tile([P, D], F32, tag="bv", name="bv_sb")
        nc.sync.dma_start(out=bv_sb, in_=bv_d.ap())
        bo_sb = misc.tile([P, D], F32, tag="bo", name="bo_sb")
        nc.sync.dma_start(out=bo_sb, in_=bo_d.ap())

        mf_sb = misc.tile([P, NKC], F32, tag="mf", name="mf_sb")
        nc.sync.dma_start(out=mf_sb, in_=maskm_d.ap())
        m1_sb = misc.tile([P, NKC], F32, tag="m1", name="m1_sb")
        nc.vector.tensor_scalar_add(m1_sb, mf_sb, -1.0)
        maskadd = misc.tile([P, NKC], F32, tag="maskadd", name="maskadd")
        nc.vector.tensor_scalar_mul(maskadd, m1_sb, 1000.0)

        # --- persistent big tensors ----------------------------------------
        KT = [pers.tile([P, TKC], dt_kv, tag=f"KT{i}", name=f"KT{i}")
              for i in range(NDIN)]
        QT = [pers.tile([P, TQ], dt_q, tag=f"QT{i}", name=f"QT{i}")
              for i in range(NDIN)]
        VA = [pers.tile([P, H * (DH + 1)], dt_kv, tag=f"VA{i}", name=f"VA{i}")
              for i in range(NKC)]
        ATT = [pers.tile([P, TQ], dt_att, tag=f"ATT{i}", name=f"ATT{i}")
               for i in range(NDIN)]
        OTU = [pers.tile([P, TQ], F32, tag=f"OTU{i}", name=f"OTU{i}")
               for i in range(NDIN)]
        zall = pers.tile([2 * (H // 2), TQ], F32, tag="zall", name="zall")
        zinv = pers.tile([2 * (H // 2), TQ], F32, tag="zinv", name="zinv")

        # ones columns of the augmented V
        for kt in range(NKC):
            va_v = VA[kt].rearrange("p (h c) -> p h c", c=DH + 1)
            nc.vector.memset(va_v[:, :, DH:DH + 1], 1.0)

        # ================= Phase 1: QKV projections ========================
        with tc.tile_pool(name="ph1", bufs=1) as ph1, \
             tc.tile_pool(name="wstr", bufs=1) as wstr, \
             tc.tile_pool(name="p1ps", bufs=1, space="PSUM") as p1ps:

            xks = []
            for j in range(NDIN):
                xk = ph1.tile([P, TKC], dt_x, tag=f"xk{j}", name=f"xk{j}")
                nc.sync.dma_start(out=xk, in_=xkT_d.ap()[j * P:(j + 1) * P, :])
                xks.append(xk)
            xqs = []
            for j in range(NDIN):
                xq = ph1.tile([P, TQ], dt_x, tag=f"xq{j}", name=f"xq{j}")
                nc.sync.dma_start(out=xq, in_=xqT_d.ap()[j * P:(j + 1) * P, :])
                xqs.append(xq)

            # V (natural layout, scattered into VA with the ones columns)
            hpb = FBV // DH  # heads per dv block
            for dv2 in range(D // FBV):
                wv = wstr.tile([P, NDIN, FBV], dt_w, tag="wv", bufs=2,
                               name=f"wv{dv2}")
                nc.sync.dma_start(
                    out=wv,
                    in_=wqkv_v[:, :, 2 * D + dv2 * FBV:2 * D + (dv2 + 1) * FBV])
                for kt in range(NKC):
                    ps = p1ps.tile([P, FBV], F32, tag="vps", bufs=2,
                                   name=f"vps{dv2}_{kt}")
                    for j in range(NDIN):
                        nc.tensor.matmul(
                            ps, mm(xks[j][:, kt * P:(kt + 1) * P]),
                            mm(wv[:, j, :]),
                            start=(j == 0), stop=(j == NDIN - 1))
                    va_v = VA[kt].rearrange("p (h c) -> p h c", c=DH + 1)
                    nc.vector.tensor_add(
                        va_v[:, dv2 * hpb:(dv2 + 1) * hpb, 0:DH],
                        ps.rearrange("p (h c) -> p h c", c=DH),
                        bv_sb[:, dv2 * FBV:(dv2 + 1) * FBV]
                        .rearrange("p (h c) -> p h c", c=DH))

            # K^T
            for dk in range(NDIN):
                wb = wstr.tile([P, NDIN, P], dt_w, tag="wblk", bufs=3,
                               name=f"wbk{dk}")
                nc.sync.dma_start(
                    out=wb, in_=wqkv_v[:, :, D + dk * P:D + (dk + 1) * P])
                for off, csz in _chunks(TKC, 512):
                    ps = p1ps.tile([P, 512], F32, tag="kps", bufs=2,
                                   name=f"kps{dk}_{off}")
                    for j in range(NDIN):
                        nc.tensor.matmul(
                            ps[:, :csz], mm(wb[:, j, :]),
                            mm(xks[j][:, off:off + csz]),
                            start=(j == 0), stop=(j == NDIN - 1))
                    nc.vector.tensor_scalar_add(
                        KT[dk][:, off:off + csz], ps[:, :csz],
                        bk_sb[:, dk:dk + 1])

            # Q^T
            for dq in range(NDIN):
                wb = wstr.tile([P, NDIN, P], dt_w, tag="wblk", bufs=3,
                               name=f"wbq{dq}")
                nc.sync.dma_start(
                    out=wb, in_=wqkv_v[:, :, dq * P:(dq + 1) * P])
                for off, csz in _chunks(TQ, 512):
                    ps = p1ps.tile([P, 512], F32, tag="kps", bufs=2,
                                   name=f"qps{dq}_{off}")
                    for j in range(NDIN):
                        nc.tensor.matmul(
                            ps[:, :csz], mm(wb[:, j, :]),
                            mm(xqs[j][:, off:off + csz]),
                            start=(j == 0), stop=(j == NDIN - 1))
                    nc.vector.tensor_scalar_add(
                        QT[dq][:, off:off + csz], ps[:, :csz],
                        bq_sb[:, dq:dq + 1])

        # ================= Phase 2: attention ==============================
        # Per head: S^T [k,q] per ktile -> exp (full-TQ-wide, amortizes the
        # ACT fixed cost) -> PV accumulate into ot [65, TQ]. Emission order
        # pipelines PE: exp(kt) runs on ACT while PE does S^T(kt+1); PV(kt)
        # then consumes exp(kt).
        with tc.tile_pool(name="ph2", bufs=1) as ph2:
            wout_sb = []
            for j in range(NDIN):
                wo = ph2.tile([P, D], dt_att, tag=f"wo{j}", name=f"wo{j}")
                nc.sync.dma_start(out=wo, in_=wout_v[:, j, :])
                wout_sb.append(wo)

            with tc.tile_pool(name="p2ps", bufs=1, space="PSUM") as p2ps:
                for h in range(H):
                    hp, s = h // 2, h % 2
                    base = s * DH

                    def st_mm(kt, _h=h, _hp=hp, _base=base):
                        stt = p2ps.tile([P, TQ], F32, tag="st",
                                        bufs=cfg["st_bufs"],
                                        name=f"st{_h}_{kt}")
                        for off, csz in _chunks(TQ, 512):
                            nc.tensor.matmul(
                                stt[:, off:off + csz],
                                mm(KT[_hp][_base:_base + DH,
                                           kt * P:(kt + 1) * P]),
                                mm(QT[_hp][_base:_base + DH, off:off + csz]),
                                start=True, stop=True)
                        return stt

                    ot = p2ps.tile([DH + 1, TQ], F32, tag="ot", bufs=1,
                                   name=f"ot{h}")
                    stt = st_mm(0)
                    for kt in range(NKC):
                        pt = ph2.tile([P, TQ], dt_p, tag="pt",
                                      bufs=cfg["pt_bufs"], name=f"pt{h}_{kt}")
                        nc.scalar.activation(
                            out=pt, in_=stt, func=EXP,
                            bias=maskadd[:, kt:kt + 1], scale=0.125)
                        if kt + 1 < NKC:
                            stt = st_mm(kt + 1)
                        for off, csz in _chunks(TQ, 512):
                            nc.tensor.matmul(
                                ot[:, off:off + csz],
                                mm(VA[kt][:, h * (DH + 1):(h + 1) * (DH + 1)]),
                                mm(pt[:, off:off + csz]),
                                start=(kt == 0), stop=(kt == NKC - 1))

                    # unnormalized att^T + Z row (engine writes must start
                    # at a 32-aligned partition: stage Z at partition 0,
                    # then DMA -- no alignment constraint -- into zall).
                    nc.vector.tensor_copy(OTU[hp][base:base + DH, :],
                                          ot[0:DH, :])
                    zt = ph2.tile([1, TQ], F32, tag="zt", bufs=4,
                                  name=f"zt{h}")
                    nc.vector.tensor_copy(zt, ot[DH:DH + 1, :])
                    nc.sync.dma_start(out=zall[h:h + 1, :], in_=zt)

            # batched 1/Z + partition-broadcast (SBUF src DMAs forbid
            # step-0 partition reads, DRAM srcs allow them -- bounce
            # through a DRAM scratch) + normalize into ATT.
            nc.vector.reciprocal(zinv, zall)
            zd = zdram.tile([H, TQ], F32, tag="zd", name="zd")
            nc.sync.dma_start(out=zd, in_=zinv)
            for hp in range(H // 2):
                zsl = zd[2 * hp:2 * hp + 2, :]
                zsrc = bass.AP(
                    tensor=zsl.tensor, offset=zsl.offset,
                    ap=[list(zsl.ap[0]), [0, DH], list(zsl.ap[1])])
                zr = ph2.tile([P, TQ], F32, tag="zr", bufs=4,
                              name=f"zr{hp}")
                nc.sync.dma_start(out=zr, in_=zsrc)
                nc.vector.tensor_mul(ATT[hp], OTU[hp], zr)

            # ============= Phase 3: output projection ======================
            with tc.tile_pool(name="ph3", bufs=1) as ph3, \
                 tc.tile_pool(name="p3ps", bufs=1, space="PSUM") as p3ps:
                for tb in range(TQ // P):
                    for dc in range(D // FBO):
                        ps = p3ps.tile([P, FBO], F32, tag="ops", bufs=2,
                                       name=f"ops{tb}_{dc}")
                        for j in range(NDIN):
                            nc.tensor.matmul(
                                ps,
                                mm(ATT[j][:, tb * P:(tb + 1) * P]),
                                mm(wout_sb[j][:, dc * FBO:(dc + 1) * FBO]),
                                start=(j == 0), stop=(j == NDIN - 1))
                        ob = ph3.tile([P, FBO], F32, tag="ob", bufs=3,
                                      name=f"ob{tb}_{dc}")
                        nc.vector.tensor_add(
                            ob, ps, bo_sb[:, dc * FBO:(dc + 1) * FBO])
                        nc.sync.dma_start(
                            out=y_d.ap()[tb * P:(tb + 1) * P,
                                         dc * FBO:(dc + 1) * FBO],
                            in_=ob)

    nc.compile()
    return nc, in_names


def shard_inputs(dims, cfg, NKC, x, mask, W_qkv, b_qkv, W_out, b_out):
    """Host-side sharding: slices, layout transposes/permutation, bias
    tiling. The key permutation puts unmasked keys first (padding keeps
    mask=0 so the device-side bias kills it)."""
    B, T, D = dims["B"], dims["T"], dims["D"]
    TQ = T // 2
    NDIN = D // P
    TKC = NKC * P
    npx = _np_dt(cfg["dt_x"])
    npw = _np_dt(cfg["dt_w"])
    npa = _np_dt(cfg["dt_att"])

    x = np.asarray(x)
    mask = np.asarray(mask)
    W_qkv = np.asarray(W_qkv)
    b_qkv = np.asarray(b_qkv)
    W_out = np.asarray(W_out)
    b_out = np.asarray(b_out)

    wqkv_c = np.ascontiguousarray(W_qkv.astype(npw))
    wout_c = np.ascontiguousarray(W_out.astype(npa))
    bq = np.ascontiguousarray(b_qkv[:D].reshape(NDIN, P).T.astype(np.float32))
    bk = np.ascontiguousarray(
        b_qkv[D:2 * D].reshape(NDIN, P).T.astype(np.float32))
    bv = np.ascontiguousarray(
        np.broadcast_to(b_qkv[2 * D:], (P, D)).astype(np.float32))
    bo = np.ascontiguousarray(
        np.broadcast_to(b_out, (P, D)).astype(np.float32))

    in_maps = []
    percore = {}
    for b in range(B):
        mb = mask[b, 0, 0]
        idx_on = np.nonzero(mb == 1)[0]
        perm = np.zeros(TKC, dtype=np.int64)  # pad with key 0 (masked off)
        perm[:len(idx_on)] = idx_on
        mc = np.zeros(TKC, dtype=np.float32)
        mc[:len(idx_on)] = 1.0
        xkT = np.ascontiguousarray(x[b][perm].T.astype(npx))
        maskm = np.ascontiguousarray(mc.reshape(NKC, P).T)
        percore[b] = (xkT, maskm)

    for c in range(2 * B):
        b, hf = c // 2, c % 2
        xkT, maskm = percore[b]
        xqT = np.ascontiguousarray(
            x[b, hf * TQ:(hf + 1) * TQ, :].T.astype(npx))
        in_maps.append(dict(
            xkT=xkT, xqT=xqT, wqkv=wqkv_c, wout=wout_c,
            bq=bq, bk=bk, bv=bv, bo=bo, maskm=maskm))
    return in_maps


_CACHE = {}
LAST_EXEC_NS = None


def kernel(x, mask, W_qkv, b_qkv, W_out, b_out):
    global LAST_EXEC_NS
    dims = FULL_DIMS
    cfg = DEFAULT_CFG
    _install_ntff_shim()

    mask = np.asarray(mask)
    counts = mask.reshape(dims["B"], -1).sum(1)
    NKC = max(1, int(np.ceil(counts.max() / P)))
    NKC = min(NKC, dims["T"] // P)

    if NKC not in _CACHE:
        _CACHE[NKC] = build_nc(dims, cfg, NKC)
    nc, _ = _CACHE[NKC]

    in_maps = shard_inputs(dims, cfg, NKC, x, mask, W_qkv, b_qkv,
                           W_out, b_out)
    trace = bool(os.environ.get("KERNEL_TRACE"))
    res = bass_utils.run_bass_kernel_spmd(
        nc, in_maps, core_ids=list(range(8)), trace=trace,
        tmpdir=os.environ.get("KERNEL_TRACE_DIR") or None)
    LAST_EXEC_NS = res.exec_time_ns

    B, T, D = dims["B"], dims["T"], dims["D"]
    TQ = T // 2
    out = np.empty((B, T, D), dtype=np.float32)
    for c in range(2 * B):
        b, hf = c // 2, c % 2
        out[b, hf * TQ:(hf + 1) * TQ, :] = res.results[c]["y"]
    return out
